# revision 1
# baseline (speedup 1.0000x reference)
"""Trainium2 Bass kernel for nn_Model_14998025797662 (Mamba-TimeVariant classifier).

Self-contained: hardcodes shapes. Data-parallel over batch: 16 samples ->
8 cores x 2 samples. Layout: channels-on-partitions, time-on-free [d_p, t].
SSM scan uses the native DVE tensor_tensor_scan (state = dA*state + dBu).
The causal depthwise conv is folded into the in_proj matmul (shifted-emb
K-tiles with host-combined weights). out_proj is folded into per-state PE
matmuls; the C contraction runs as a fused scalar_tensor_tensor chain in
[t_p, m] space. B broadcasting uses stride-0-partition DMA reads via a DRAM
bounce. bf16 is used on the additive scan path (dBu, h, gates) which the
final 1024-token averaging renders harmless; decays dA stay fp32.
"""

import numpy as np

import concourse.bacc as bacc
import concourse.bass as bass
from concourse import mybir
from concourse.bass import ds, ts
from concourse.tile import TileContext

F32 = mybir.dt.float32
BF16 = mybir.dt.bfloat16
AF = mybir.ActivationFunctionType
OP = mybir.AluOpType
AX = mybir.AxisListType

B, L, CIN = 16, 1024, 12
DM, DS, DC, DI, DTR = 256, 16, 4, 512, 16
NCLS, NH, EK = 10, 8, 3
NCORES = 8
BLOC = B // NCORES          # 2 samples per core
NDT = DI // 128             # 4 d-tiles
NTCH = L // 128             # 8 time chunks
KD = EK * CIN               # 36
PAD = DC - 1                # 3 left pad cols for causal conv


def _patch_act_tables():
    """Bias ACT table-set selection: the rust pass picks the first set
    containing each function, so Exp and Ln land in different sets and the
    dt-softplus + LN stages thrash table loads (~2.7us each). Strip Exp/Ln
    from the single-function sets so both resolve to
    natural_log_exp_and_others. Idempotent."""
    import concourse.bacc as _bacc
    import concourse.hw_specs as _hw
    if getattr(_bacc, "_ant_act_tables_patched", False):
        return
    _orig = _hw.get_activation_tables

    def patched(arch):
        t = _orig(arch)
        both = None
        for name, fns in t.items():
            sn = {str(x).split(".")[-1] for x in fns}
            if "Exp" in sn and "Ln" in sn:
                both = name
                break
        if both is not None:
            for name, fns in t.items():
                if name == both:
                    continue
                fns.discard(mybir.ActivationFunctionType.Exp)
                fns.discard(mybir.ActivationFunctionType.Ln)
        return t

    _bacc.get_activation_tables = patched
    _bacc._ant_act_tables_patched = True


DEBUG_STOP = None  # None | "A" | "B<k>" (stop after k states) | "C1"


def _build_module():
    _patch_act_tables()
    nc = bacc.Bacc("TRN2", target_bir_lowering=False)

    def din(name, shape, dt=F32):
        return nc.dram_tensor(name, shape, dt, kind="ExternalInput")

    xT = din("xT", [BLOC, CIN, L], BF16)
    xmark = din("xmark", [BLOC, L])
    tok_lhsT = din("tok_lhsT", [KD, DM], BF16)
    peT = din("peT", [DM, L])
    inWzT = din("inWzT", [DM, DI], BF16)    # z half of in_proj
    convWT = din("convWT", [DC * DM, DI], BF16)
    dcb = din("dcb", [DI, 1])
    xprojWT = din("xprojWT", [DI, DTR + 2 * DS], BF16)
    dtWT = din("dtWT", [DTR, DI], BF16)
    dtb = din("dtb", [DI, 1])
    Amat = din("Amat", [DI, DS])
    Dv = din("Dv", [DI, 1])
    WoutT = din("WoutT", [DI, DM], BF16)
    lng_bc = din("lng_bc", [128, DM])
    lnb_bc = din("lnb_bc", [128, DM])
    headWT = din("headWT", [DM, NCLS + NH], BF16)
    attnb_bc = din("attnb_bc", [128, NH])
    onec = din("onec", [128, 1])
    epsc = din("epsc", [128, 1])
    ident = din("ident", [128, 128], BF16)

    out = nc.dram_tensor("out", [BLOC, NCLS], F32, kind="ExternalOutput")
    scr_am = nc.dram_tensor("scr_am", [BLOC, L], F32)
    scr_wx = nc.dram_tensor("scr_wx", [BLOC, L], F32)
    scr_br = nc.dram_tensor("scr_br", [BLOC, DS, L], BF16)
    scr_cr = nc.dram_tensor("scr_cr", [BLOC, DS, L], BF16)

    with TileContext(nc) as tc:
        with (
            tc.tile_pool(name="const", bufs=1) as cp,
            tc.tile_pool(name="persist", bufs=1) as pp,
            tc.tile_pool(name="work", bufs=2) as wp,
            tc.tile_pool(name="small", bufs=2) as sp,
            tc.tile_pool(name="psumr", bufs=3, space="PSUM") as psr,
        ):
            def cload(name, shape, src, dt=F32):
                t = cp.tile(shape, dt, name=f"c_{name}")
                nc.sync.dma_start(t[:], src)
                return t

            tokW_sb = cload("tokW", [KD, DM], tok_lhsT[:], BF16)
            inWz_sb = [cload(f"inWz{k}", [128, DI], inWzT[ts(k, 128), :], BF16) for k in range(2)]
            convW_sb = [cload(f"cvW{k}", [128, DI], convWT[ts(k, 128), :], BF16) for k in range(8)]
            dcb_sb = [cload(f"dcb{d}", [128, 1], dcb[ts(d, 128), :]) for d in range(NDT)]
            xprojW_sb = [cload(f"xpW{d}", [128, DTR + 2 * DS], xprojWT[ts(d, 128), :], BF16) for d in range(NDT)]
            dtW_sb = cload("dtW", [DTR, DI], dtWT[:], BF16)
            dtb_sb = [cload(f"dtb{d}", [128, 1], dtb[ts(d, 128), :]) for d in range(NDT)]
            A_sb = [cload(f"A{d}", [128, DS], Amat[ts(d, 128), :]) for d in range(NDT)]
            Dv_sb = [cload(f"Dv{d}", [128, 1], Dv[ts(d, 128), :]) for d in range(NDT)]
            Wout_sb = [cload(f"Wo{d}", [128, DM], WoutT[ts(d, 128), :], BF16) for d in range(NDT)]
            lng_sb = cload("lng", [128, DM], lng_bc[:])
            lnb_sb = cload("lnb", [128, DM], lnb_bc[:])
            headW_sb = [cload(f"hW{k}", [128, NCLS + NH], headWT[ts(k, 128), :], BF16) for k in range(2)]
            attnb_sb = cload("attnb", [128, NH], attnb_bc[:])
            one_sb = cload("onec", [128, 1], onec[:])
            eps_sb = cload("epsc", [128, 1], epsc[:])
            id_sb = cload("ident", [128, 128], ident[:], BF16)

            for b in range(BLOC):
                # ======== stage A ========
                rhs36 = pp.tile([KD, L], BF16, name=f"rhs36_{b}", tag="rhs36")
                nc.sync.dma_start(rhs36[12:24, :], xT[b, :, :])
                nc.sync.dma_start(rhs36[0:12, 1:L], xT[b, :, 0:L - 1])
                nc.sync.dma_start(rhs36[0:12, 0:1], xT[b, :, 0:1])
                nc.sync.dma_start(rhs36[24:36, 0:L - 1], xT[b, :, 1:L])
                nc.sync.dma_start(rhs36[24:36, L - 1:L], xT[b, :, L - 1:L])

                pe_sb = []
                for k in range(2):
                    pt = wp.tile([128, L], F32, name=f"pe{k}_{b}", tag="wD", bufs=3)
                    nc.sync.dma_start(pt[:], peT[ts(k, 128), :])
                    pe_sb.append(pt)

                # emb with PAD left zero cols (for shifted conv reads)
                emb_sb = [pp.tile([128, PAD + L], BF16, name=f"emb{m}_{b}", tag=f"emb{m}") for m in range(2)]
                for m in range(2):
                    nc.vector.memset(emb_sb[m][:, 0:PAD], 0.0)
                    for n in range(2):
                        eps_ps = psr.tile([128, 512], F32, name=f"eps{m}{n}_{b}", tag="ps512")
                        nc.tensor.matmul(eps_ps[:], tokW_sb[:, ts(m, 128)], rhs36[:, ts(n, 512)],
                                         start=True, stop=True)
                        nc.vector.tensor_add(emb_sb[m][:, ds(PAD + n * 512, 512)], eps_ps[:],
                                             pe_sb[m][:, ts(n, 512)])

                # z half of in_proj -> silu(z) in bf16
                sz_sb = [pp.tile([128, L], BF16, name=f"sz{d}_{b}", tag=f"sz{d}") for d in range(NDT)]
                for d in range(NDT):
                    for n in range(2):
                        ps = psr.tile([128, 512], F32, name=f"z{d}{n}_{b}", tag="ps512")
                        for k in range(2):
                            nc.tensor.matmul(ps[:], inWz_sb[k][:, ts(d, 128)],
                                             emb_sb[k][:, ds(PAD + n * 512, 512)],
                                             start=(k == 0), stop=(k == 1))
                        nc.scalar.activation(sz_sb[d][:, ts(n, 512)], ps[:], AF.Silu)

                # fused conv(in_proj x-half) -> u0 = silu(. + dconv_b)
                u0_sb = [pp.tile([128, L], BF16, name=f"u0{d}_{b}", tag=f"u0{d}") for d in range(NDT)]
                for d in range(NDT):
                    for n in range(2):
                        ps = psr.tile([128, 512], F32, name=f"u{d}{n}_{b}", tag="ps512")
                        for k in range(8):
                            j = k // 2
                            nc.tensor.matmul(ps[:], convW_sb[k][:, ts(d, 128)],
                                             emb_sb[k % 2][:, ds(j + n * 512, 512)],
                                             start=(k == 0), stop=(k == 7))
                        nc.scalar.activation(u0_sb[d][:, ts(n, 512)], ps[:], AF.Silu,
                                             bias=dcb_sb[d][:, 0:1])

                # x_proj -> x_dblT [48, L]; B rows cast to bf16 + DRAM stage
                xdbl_sb = pp.tile([DTR + 2 * DS, L], BF16, name=f"xdbl_{b}", tag="rhs36")
                for n in range(2):
                    ps = psr.tile([48, 512], F32, name=f"xd{n}_{b}", tag="ps512")
                    for k in range(NDT):
                        nc.tensor.matmul(ps[:], xprojW_sb[k][:], u0_sb[k][:, ts(n, 512)],
                                         start=(k == 0), stop=(k == NDT - 1))
                    nc.scalar.copy(xdbl_sb[:, ts(n, 512)], ps[:])
                for s in range(DS):
                    nc.sync.dma_start(scr_br[b, s, :], xdbl_sb[DTR + DS + s:DTR + DS + s + 1, :])
                    nc.sync.dma_start(scr_cr[b, s, :], xdbl_sb[DTR + s:DTR + s + 1, :])

                # dt = ln(1 + exp(pre + bias))
                dt_sb = [pp.tile([128, L], F32, name=f"dt{d}_{b}", tag=f"dt{d}") for d in range(NDT)]
                for d in range(NDT):
                    for n in range(2):
                        ps = psr.tile([128, 512], F32, name=f"dtp{d}{n}_{b}", tag="ps512")
                        nc.tensor.matmul(ps[:], dtW_sb[:, ts(d, 128)], xdbl_sb[0:DTR, ts(n, 512)],
                                         start=True, stop=True)
                        esp = wp.tile([128, 512], F32, name=f"esp{d}{n}_{b}", tag="cacc")
                        nc.scalar.activation(esp[:], ps[:], AF.Exp, bias=dtb_sb[d][:, 0:1])
                        nc.scalar.activation(dt_sb[d][:, ts(n, 512)], esp[:], AF.Ln,
                                             bias=one_sb[:, 0:1])

                # w = dt * u (bf16)
                wT_sb = [pp.tile([128, L], BF16, name=f"w{d}_{b}", tag=f"w{d}") for d in range(NDT)]
                for d in range(NDT):
                    nc.vector.tensor_mul(wT_sb[d][:], dt_sb[d][:], u0_sb[d][:])

                if DEBUG_STOP == "A":
                    nc.sync.dma_start(out[b, :], xdbl_sb[0:1, 0:NCLS])
                    continue
                # ======== stage B: 16 SSM states, C-contraction in d-space ========
                nstates = DS if not (DEBUG_STOP or "").startswith("B") else int(DEBUG_STOP[1:])
                acc = [None] * NDT
                for s in range(nstates):
                    bbc = wp.tile([128, L], BF16, name=f"bbc{s}_{b}", tag="bbc")
                    nc.sync.dma_start(bbc[:], scr_br[b, s:s + 1, :].to_broadcast((128, L)))
                    cbc = wp.tile([128, L], BF16, name=f"cbc{s}_{b}", tag="cbc")
                    nc.sync.dma_start(cbc[:], scr_cr[b, s:s + 1, :].to_broadcast((128, L)))
                    for d in range(NDT):
                        dA = wp.tile([128, L], F32, name=f"dA{s}{d}_{b}", tag="wB", bufs=3)
                        nc.scalar.activation(dA[:], dt_sb[d][:], AF.Exp, scale=A_sb[d][:, s:s + 1])
                        dBu = wp.tile([128, L], BF16, name=f"dBu{s}{d}_{b}", tag="wC", bufs=3)
                        nc.vector.tensor_mul(dBu[:], wT_sb[d][:], bbc[:])
                        h = wp.tile([128, L], BF16, name=f"h{s}{d}_{b}", tag="wD", bufs=3)
                        nc.vector.tensor_tensor_scan(h[:], dA[:], dBu[:], 0.0,
                                                     op0=OP.mult, op1=OP.add)
                        if s == 0:
                            a2 = wp.tile([128, L], BF16, name=f"acc{s}{d}_{b}", tag=f"acc{d}")
                            nc.vector.tensor_mul(a2[:], h[:], cbc[:])
                        else:
                            term = wp.tile([128, L], BF16, name=f"term{s}{d}_{b}", tag="wE", bufs=3)
                            nc.vector.tensor_mul(term[:], h[:], cbc[:])
                            a2 = wp.tile([128, L], BF16, name=f"acc{s}{d}_{b}", tag=f"acc{d}")
                            nc.vector.tensor_add(a2[:], acc[d][:], term[:])
                        acc[d] = a2

                # ytot = (acc + u*D) * sz  (bf16)
                ytot = []
                for d in range(NDT):
                    t1 = wp.tile([128, L], BF16, name=f"yt1{d}_{b}", tag="wE", bufs=3)
                    nc.vector.scalar_tensor_tensor(t1[:], u0_sb[d][:], Dv_sb[d][:, 0:1],
                                                   acc[d][:], op0=OP.mult, op1=OP.add)
                    yt = wp.tile([128, L], BF16, name=f"ytot{d}_{b}", tag=f"acc{d}")
                    nc.vector.tensor_mul(yt[:], t1[:], sz_sb[d][:])
                    ytot.append(yt)

                # out_proj: mo[t, m] accumulated in PSUM per time chunk
                mo_ps = []
                for t in range(NTCH):
                    mp = psr.tile([128, DM], F32, name=f"mo{t}_{b}", tag="MO", bufs=4)
                    for d in range(NDT):
                        nc.tensor.matmul(mp[:], ytot[d][:, ts(t, 128)], Wout_sb[d][:],
                                         start=(d == 0), stop=(d == NDT - 1))
                    mo_ps.append(mp)
                macc = []
                for t in range(NTCH):
                    mosb = wp.tile([128, DM], F32, name=f"mosb{t}_{b}", tag=f"macc{t}")
                    ssum = sp.tile([128, 1], F32, name=f"ssum{t}_{b}", tag=f"ssum{t}")
                    nc.scalar.activation(mosb[:], mo_ps[t][:], AF.Identity, accum_out=ssum[:])
                    macc.append((mosb, ssum))

                if (DEBUG_STOP or "").startswith("B"):
                    for t in range(NTCH):
                        mo, ssum = macc[t]
                        nc.sync.dma_start(out[b, :], mo[0:1, 0:NCLS])
                    continue
                # ======== stage C ========
                t2_all = []
                for t in range(NTCH):
                    mo, ssum = macc[t]
                    sq = wp.tile([128, DM], F32, name=f"sq{t}_{b}", tag="wC", bufs=3)
                    sqs = sp.tile([128, 1], F32, name=f"sqs{t}_{b}", tag="sqs")
                    nc.vector.scalar_tensor_tensor(sq[:], mo[:], 1.0, mo[:],
                                                   op0=OP.mult, op1=OP.mult, accum_out=sqs[:])
                    mun = sp.tile([128, 1], F32, name=f"mun{t}_{b}", tag="mun")
                    nc.vector.tensor_scalar_mul(mun[:], ssum[:], -1.0 / DM)
                    m2t = sp.tile([128, 1], F32, name=f"m2t{t}_{b}", tag="m2t")
                    nc.vector.tensor_scalar_mul(m2t[:], sqs[:], 1.0 / DM)
                    msq = sp.tile([128, 1], F32, name=f"msq{t}_{b}", tag="msq")
                    nc.vector.tensor_mul(msq[:], mun[:], mun[:])
                    var = sp.tile([128, 1], F32, name=f"var{t}_{b}", tag="var")
                    nc.vector.tensor_sub(var[:], m2t[:], msq[:])
                    # rstd = exp(-0.5*ln(var+eps)) keeps everything in the ln/exp set
                    lnv = sp.tile([128, 1], F32, name=f"lnv{t}_{b}", tag="lnv")
                    nc.scalar.activation(lnv[:], var[:], AF.Ln, bias=eps_sb[:, 0:1])
                    rstd = sp.tile([128, 1], F32, name=f"rstd{t}_{b}", tag="rstd")
                    nc.scalar.activation(rstd[:], lnv[:], AF.Exp, scale=-0.5)
                    nmr = sp.tile([128, 1], F32, name=f"nmr{t}_{b}", tag="nmr")
                    nc.vector.tensor_mul(nmr[:], mun[:], rstd[:])
                    xn = wp.tile([128, DM], F32, name=f"xn{t}_{b}", tag="cacc")
                    nc.scalar.activation(xn[:], mo[:], AF.Identity,
                                         bias=nmr[:, 0:1], scale=rstd[:, 0:1])
                    t1 = wp.tile([128, DM], F32, name=f"t1{t}_{b}", tag="wC", bufs=3)
                    nc.vector.tensor_mul(t1[:], xn[:], lng_sb[:])
                    t2 = wp.tile([128, DM], F32, name=f"t2{t}_{b}", tag=f"macc{t}")
                    nc.vector.tensor_add(t2[:], t1[:], lnb_sb[:])
                    t2_all.append(t2)
                mam_sb = []
                for t in range(NTCH):
                    mam = pp.tile([128, DM], BF16, name=f"mam{t}_{b}", tag=f"mam{t}")
                    nc.scalar.activation(mam[:], t2_all[t][:], AF.Silu)
                    mam_sb.append(mam)

                if DEBUG_STOP == "C1":
                    nc.sync.dma_start(out[b, :], mam_sb[0][0:1, 0:NCLS])
                    continue
                moT = [pp.tile([128, L], BF16, name=f"moT{m}_{b}", tag=f"moT{m}") for m in range(2)]
                for t in range(NTCH):
                    for m in range(2):
                        tp = psr.tile([128, 128], BF16, name=f"mtp{t}{m}_{b}", tag="ps512")
                        nc.tensor.transpose(tp[:], mam_sb[t][:, ts(m, 128)], id_sb[:])
                        nc.scalar.copy(moT[m][:, ts(t, 128)], tp[:])

                logit_sb = []
                for t in range(NTCH):
                    ps = psr.tile([128, NCLS + NH], F32, name=f"hd{t}_{b}", tag="ps512")
                    for k in range(2):
                        nc.tensor.matmul(ps[:], moT[k][:, ts(t, 128)], headW_sb[k][:],
                                         start=(k == 0), stop=(k == 1))
                    lg = pp.tile([128, NCLS], F32, name=f"lg{t}_{b}", tag=f"lg{t}")
                    nc.scalar.copy(lg[:], ps[:, 0:NCLS])
                    logit_sb.append(lg)
                    atb = wp.tile([128, NH], F32, name=f"atb{t}_{b}", tag="atb")
                    nc.vector.tensor_add(atb[:], ps[:, NCLS:NCLS + NH], attnb_sb[:])
                    am = sp.tile([128, 1], F32, name=f"am{t}_{b}", tag="am")
                    nc.vector.reduce_max(am[:], atb[:], axis=AX.X)
                    nc.sync.dma_start(scr_am[b, ts(t, 128)], am[:])

                if DEBUG_STOP == "C2":
                    nc.sync.dma_start(out[b, :], logit_sb[0][0:1, 0:NCLS])
                    continue
                row = sp.tile([1, L], F32, name=f"row_{b}", tag="rA", bufs=1)
                nc.sync.dma_start(row[:], scr_am[b, :])
                mx = sp.tile([1, 1], F32, name=f"mx_{b}", tag="mx")
                nc.vector.reduce_max(mx[:], row[:], axis=AX.X)
                nmx = sp.tile([1, 1], F32, name=f"nmx_{b}", tag="nmx")
                nc.vector.tensor_scalar_mul(nmx[:], mx[:], -1.0)
                ex = sp.tile([1, L], F32, name=f"ex_{b}", tag="rB", bufs=1)
                esum = sp.tile([1, 1], F32, name=f"esum_{b}", tag="esum")
                nc.scalar.activation(ex[:], row[:], AF.Exp, bias=nmx[0:1, 0:1],
                                     accum_out=esum[:])
                rec = sp.tile([1, 1], F32, name=f"rec_{b}", tag="rec")
                nc.vector.reciprocal(rec[:], esum[:])
                wsm = sp.tile([1, L], F32, name=f"wsm_{b}", tag="rA", bufs=1)
                nc.vector.tensor_scalar_mul(wsm[:], ex[:], rec[0:1, 0:1])
                xmrow = sp.tile([1, L], F32, name=f"xmr_{b}", tag="rC", bufs=1)
                nc.sync.dma_start(xmrow[:], xmark[b, :])
                wx = sp.tile([1, L], F32, name=f"wx_{b}", tag="rB", bufs=1)
                nc.vector.tensor_mul(wx[:], wsm[:], xmrow[:])
                nc.sync.dma_start(scr_wx[b, :], wx[:])

                if DEBUG_STOP == "C3":
                    nc.sync.dma_start(out[b, :], wx[0:1, 0:NCLS])
                    continue
                ops = psr.tile([NCLS, 1], F32, name=f"ops_{b}", tag="ps512")
                wxc = []
                for t in range(NTCH):
                    wc = sp.tile([128, 1], F32, name=f"wxc{t}_{b}", tag=f"wxc{t}")
                    nc.sync.dma_start(wc[:], scr_wx[b, ts(t, 128)])
                    wxc.append(wc)
                for t in range(NTCH):
                    nc.tensor.matmul(ops[:], logit_sb[t][:], wxc[t][:],
                                     start=(t == 0), stop=(t == NTCH - 1))
                oc = sp.tile([NCLS, 1], F32, name=f"oc_{b}", tag="oc")
                nc.vector.tensor_copy(oc[:], ops[:])
                nc.sync.dma_start(out[b, :], oc[:])

    nc.finalize()
    return nc


_NC_CACHE = None


def _get_module():
    global _NC_CACHE
    if _NC_CACHE is None:
        _NC_CACHE = _build_module()
    return _NC_CACHE


def _pos_emb_T():
    pos = np.arange(L, dtype=np.float32)[:, None]
    div = np.exp(np.arange(0, DM, 2, dtype=np.float32) * (-np.log(10000.0) / DM))
    pe = np.zeros((L, DM), np.float32)
    pe[:, 0::2] = np.sin(pos * div)
    pe[:, 1::2] = np.cos(pos * div)
    return pe.T.copy()


def _prep_inputs(inputs):
    import ml_dtypes
    f = lambda x: np.ascontiguousarray(np.asarray(x, dtype=np.float32))
    tokW = f(inputs["tok_conv_w"])                        # [DM, CIN, EK]
    inW = f(inputs["in_proj_w"])                          # [2DI, DM]
    cvw = f(inputs["dconv_w"])[:, 0, :]                   # [DI, DC]
    # convWT[(j,m), d] = in_proj_w[d, m] * dconv_w[d, j]
    convWT = (inW[:DI][None, :, :] * cvw.T[:, :, None]).transpose(0, 2, 1)  # [DC, DM, DI]
    convWT = np.ascontiguousarray(convWT.reshape(DC * DM, DI))
    bf = lambda x: np.ascontiguousarray(x).astype(ml_dtypes.bfloat16)
    shared = {
        "tok_lhsT": bf(np.transpose(tokW, (2, 1, 0)).reshape(KD, DM)),
        "peT": _pos_emb_T(),
        "inWzT": bf(inW[DI:].T),
        "convWT": bf(convWT),
        "dcb": f(inputs["dconv_b"]).reshape(DI, 1),
        "xprojWT": bf(f(inputs["x_proj_w"]).T[:, list(range(DTR)) + list(range(DTR + DS, DTR + 2 * DS)) + list(range(DTR, DTR + DS))]),
        "dtWT": bf(f(inputs["dt_proj_w"]).T),
        "dtb": f(inputs["dt_proj_b"]).reshape(DI, 1),
        "Amat": (-np.exp(f(inputs["A_log"]))).astype(np.float32),
        "Dv": f(inputs["Dvec"]).reshape(DI, 1),
        "WoutT": f(inputs["out_proj_w"]).T.astype(ml_dtypes.bfloat16).copy(),
        "lng_bc": np.broadcast_to(f(inputs["ln_g"]), (128, DM)).copy(),
        "lnb_bc": np.broadcast_to(f(inputs["ln_b"]), (128, DM)).copy(),
        "headWT": bf(np.concatenate([f(inputs["cls_w"]).T, f(inputs["attn_w"]).T], axis=1)),
        "attnb_bc": np.broadcast_to(f(inputs["attn_b"]), (128, NH)).copy(),
        "onec": np.ones((128, 1), np.float32),
        "epsc": np.full((128, 1), 1e-5, np.float32),
        "ident": np.eye(128, dtype=ml_dtypes.bfloat16),
    }
    xTall = np.ascontiguousarray(f(inputs["x_enc"]).transpose(0, 2, 1))  # [B, CIN, L]
    xm = f(inputs["x_mark_enc"])
    per_core = []
    for c in range(NCORES):
        m = dict(shared)
        m["xT"] = np.ascontiguousarray(xTall[c * BLOC:(c + 1) * BLOC]).astype(ml_dtypes.bfloat16)
        m["xmark"] = np.ascontiguousarray(xm[c * BLOC:(c + 1) * BLOC])
        per_core.append(m)
    return per_core


def kernel(**inputs) -> np.ndarray:
    from concourse.bass_utils import run_bass_kernel_spmd

    nc = _get_module()
    in_maps = _prep_inputs(inputs)
    res = run_bass_kernel_spmd(nc, in_maps, core_ids=list(range(NCORES)))
    return np.concatenate([res.results[c]["out"] for c in range(NCORES)], axis=0)

